# revision 1
# baseline (speedup 1.0000x reference)
"""Trainium2 Bass kernel for nn_EnhancedQuantumLLM.

Math (B=2, H=16, L=1024, D=64, LMAX=2048):
  The per-scale pattern multiply is a per-(h,l) complex scalar c_l, so
  scores S = Qp @ Kp^T = c_l c_m S0 with S0 = Q @ K^T (complex) computed
  once per (b,h).  mag = |c_l||c_m||S0|/sqrt(D).  The softmax argument
  x = a_l a_m |S0|/8 is tiny (<= ~0.012), so exp(x) = 1 + x to ~1e-7 and
  softmax(w) = (1 + x)/ (L + sum x).  The "1" is handled exactly via the
  fp32 column-sum of V accumulated into the same PSUM group, keeping the
  bf16 matmuls operating only on the small signal x.

Sharding: 32 (b,h) pairs over 8 cores; core c owns h in {2c, 2c+1}, b in
{0,1}.  Patterns are input-independent and precomputed on host.
"""
import sys

for _p in ("/opt/trn_rl_repo",):
    if _p not in sys.path:
        sys.path.insert(0, _p)

import numpy as np
import ml_dtypes

B, H, L, D = 2, 16, 1024, 64
LMAX = 2048
PI = float(np.pi)
N_CORES = 8
PAIRS = [(0, 0), (0, 1), (1, 0), (1, 1)]  # (b, h_local)
NMC = L // 128  # m-chunks
NLC = L // 128  # l-chunks
VW = 2 * D + 1  # Vpack width: [Vr | Vi | ones]
PW = VW + 1     # padded width so two f-halves pack into one matmul rhs
BF16 = ml_dtypes.bfloat16

_module_cache = {}


# ---------------------------------------------------------------- host math
def _scale_abs():
    """|c^f[h,l]| for the 4 scale freqs, [4, H, L] float64->float32."""
    out = np.empty((4, H, L), np.float64)
    for fi, freq in enumerate([1.0, 0.5, 0.25, 0.1]):
        phase = 2.0 * PI * np.arange(H, dtype=np.float64) / H
        t = np.linspace(0.0, 2.0 * PI * freq, LMAX)
        a1 = t[None, :] + phase[:, None]
        a2 = 2.0 * t[None, :] + phase[:, None]
        a3 = 0.5 * t[None, :] + phase[:, None]
        pr = np.cos(a1) + np.cos(a2) + np.cos(a3)
        pi_ = np.sin(a1) + np.sin(a2) + np.sin(a3)
        norm = np.sqrt(np.sum(pr * pr + pi_ * pi_, axis=1, keepdims=True))
        pr, pi_ = pr / norm, pi_ / norm
        out[fi] = np.sqrt(pr * pr + pi_ * pi_)[:, :L]
    return out.astype(np.float32)


def _expert_quad():
    """[128, NLC, 256] f32: [epr|epi|epi|epr] per l-chunk, x0.5 folded."""
    freqs = np.array([[0.3 + 0.1 * i, 0.2 + 0.1 * i, 0.1 + 0.1 * i]
                      for i in range(8)], np.float64).reshape(-1)
    t = np.linspace(0.0, 2.0 * PI, LMAX)
    phase_d = 2.0 * PI * np.arange(D, dtype=np.float64) / D
    ang = freqs[:, None, None] * t[None, :, None] + phase_d[None, None, :]
    col_norm = 1.0 / np.sqrt(float(LMAX))
    denom = np.sqrt(3.0) * np.sqrt(8.0)
    epr = (np.sum(np.cos(ang), axis=0) * (col_norm / denom))[:L] * 0.5
    epi = (np.sum(np.sin(ang), axis=0) * (col_norm / denom))[:L] * 0.5
    quad = np.concatenate([epr, epi, epi, epr], axis=1)  # [L, 256]
    return np.ascontiguousarray(
        quad.reshape(NLC, 128, 4 * D).transpose(1, 0, 2)).astype(np.float32)


# ---------------------------------------------------------------- device code
def _build_module():
    import concourse.bacc as bacc
    import concourse.tile as tile
    from concourse import mybir

    dt = mybir.dt
    op = mybir.AluOpType
    AF = mybir.ActivationFunctionType

    nc = bacc.Bacc("TRN2", target_bir_lowering=False, debug=False,
                   num_devices=N_CORES)

    qa_d = nc.dram_tensor("qa", [4, 128, L], dt.bfloat16, kind="ExternalInput").ap()
    qb_d = nc.dram_tensor("qb", [4, 128, L], dt.bfloat16, kind="ExternalInput").ap()
    kt_d = nc.dram_tensor("kt", [4, 128, L], dt.bfloat16, kind="ExternalInput").ap()
    vp_d = nc.dram_tensor("vp", [4, 128, NMC, PW], dt.bfloat16, kind="ExternalInput").ap()
    vf_d = nc.dram_tensor("vf", [4, 128, NMC, VW], dt.float32, kind="ExternalInput").ap()
    aiv_d = nc.dram_tensor("aiv", [6, 4096], dt.bfloat16, kind="ExternalInput").ap()
    ams_d = nc.dram_tensor("ams", [128, 64], dt.float32, kind="ExternalInput").ap()
    epq_d = nc.dram_tensor("epq", [128, NLC, 4 * D], dt.float32, kind="ExternalInput").ap()
    out_d = nc.dram_tensor("out", [4, 2, NLC, 128, D], dt.float32,
                           kind="ExternalOutput").ap()

    with tile.TileContext(nc) as tc:
        with (
            tc.tile_pool(name="singles", bufs=1) as singles,
            tc.tile_pool(name="qk", bufs=2) as qk,
            tc.tile_pool(name="vpool", bufs=2) as vpool,
            tc.tile_pool(name="tpool", bufs=3) as tpool,
            tc.tile_pool(name="zpool", bufs=3) as zpool,
            tc.tile_pool(name="magpool", bufs=2) as magpool,
            tc.tile_pool(name="vprime", bufs=2) as vprime,
            tc.tile_pool(name="accpool", bufs=2) as accpool,
            tc.tile_pool(name="svpool", bufs=2) as svpool,
            tc.tile_pool(name="rspool", bufs=16) as rspool,
            tc.tile_pool(name="ppool", bufs=8) as ppool,
            tc.tile_pool(name="outpool", bufs=8) as outpool,
            tc.tile_pool(name="ps_sc", bufs=1, space="PSUM") as ps_sc,
            tc.tile_pool(name="ps_av", bufs=2, space="PSUM") as ps_av,
        ):
            aiv_t = singles.tile([6, 4096], dt.bfloat16)
            nc.sync.dma_start(out=aiv_t, in_=aiv_d)
            ams_t = singles.tile([128, 64], dt.float32)
            nc.sync.dma_start(out=ams_t, in_=ams_d)
            epq_t = singles.tile([128, NLC, 4 * D], dt.float32)
            nc.sync.dma_start(out=epq_t, in_=epq_d)
            ones_col = singles.tile([128, 1], dt.float32)
            nc.vector.memset(ones_col, 1.0)

            def load_and_scores(p):
                """DMA inputs, colsum S_V, transposed scores -> mag (bf16)."""
                kt_s = qk.tile([128, L], dt.bfloat16, tag="kt_s")
                qa_s = qk.tile([128, L], dt.bfloat16, tag="qa_s")
                qb_s = qk.tile([128, L], dt.bfloat16, tag="qb_s")
                for nh in range(2):
                    sl = slice(nh * 512, (nh + 1) * 512)
                    nc.sync.dma_start(out=kt_s[:, sl], in_=kt_d[p][:, sl])
                    nc.sync.dma_start(out=qa_s[:, sl], in_=qa_d[p][:, sl])
                    nc.sync.dma_start(out=qb_s[:, sl], in_=qb_d[p][:, sl])
                vp_s = vpool.tile([128, NMC, PW], dt.bfloat16, tag="vp_s")
                nc.sync.dma_start(out=vp_s, in_=vp_d[p])
                vf_s = vpool.tile([128, NMC, VW], dt.float32, tag="vf_s")
                nc.sync.dma_start(out=vf_s, in_=vf_d[p])

                mag = magpool.tile([128, NMC, L], dt.bfloat16)
                for mc in range(NMC):
                    ps_r = ps_sc.tile([128, L], dt.float32, tag="ps_r")
                    ps_i = ps_sc.tile([128, L], dt.float32, tag="ps_i")
                    lhs = kt_s[:, mc * 128:(mc + 1) * 128]
                    for nh in range(2):
                        sl = slice(nh * 512, (nh + 1) * 512)
                        nc.tensor.matmul(ps_r[:, sl], lhs, qa_s[:, sl],
                                         start=True, stop=True)
                        nc.tensor.matmul(ps_i[:, sl], lhs, qb_s[:, sl],
                                         start=True, stop=True)
                    t1 = tpool.tile([128, L], dt.bfloat16, tag="t1")
                    nc.scalar.activation(t1, ps_r, AF.Square)
                    t2 = tpool.tile([128, L], dt.bfloat16, tag="t2")
                    nc.scalar.activation(t2, ps_i, AF.Square)
                    if mc % 2 == 0:
                        z2 = zpool.tile([128, 2, L], dt.bfloat16)
                    nc.vector.tensor_tensor(z2[:, mc % 2, :], t1, t2, op.add)
                    if mc % 2 == 1:
                        # one sqrt per chunk pair amortizes the ~350-cycle
                        # ACT per-op overhead (ACT is the bottleneck engine)
                        nc.scalar.activation(mag[:, mc - 1:mc + 1, :], z2,
                                             AF.Sqrt)

                # column sums of Vpack in fp32 (the softmax "+1" carrier row)
                sv_ps = ps_av.tile([1, VW], dt.float32, tag="of0")
                for mc in range(NMC):
                    nc.tensor.matmul(sv_ps, ones_col, vf_s[:, mc, :],
                                     start=(mc == 0), stop=(mc == NMC - 1))
                sv_s = svpool.tile([1, VW], dt.float32, tag="sv_s")
                nc.scalar.copy(sv_s, sv_ps)
                sv_hi = svpool.tile([1, VW], dt.bfloat16, tag="sv_hi")
                nc.scalar.copy(sv_hi, sv_ps)
                sv_lo = svpool.tile([1, VW], dt.bfloat16, tag="sv_lo")
                nc.vector.tensor_tensor(sv_lo, sv_s, sv_hi, op.subtract)
                # block-diagonal [6, 2*PW] rhs so one K=6 matmul seeds both
                # f-halves of the paired PSUM tile; engines can't write at
                # partition base>0, so rows are assembled via SBUF->SBUF DMA
                svr2 = svpool.tile([6, 2 * PW], dt.bfloat16, tag="svr2")
                nc.vector.memset(svr2, 0.0)
                nc.sync.dma_start(out=svr2[0:1, 0:VW], in_=sv_hi)
                nc.sync.dma_start(out=svr2[1:2, 0:VW], in_=sv_lo)
                nc.sync.dma_start(out=svr2[2:3, 0:VW], in_=sv_hi)
                nc.sync.dma_start(out=svr2[3:4, PW:PW + VW], in_=sv_hi)
                nc.sync.dma_start(out=svr2[4:5, PW:PW + VW], in_=sv_lo)
                nc.sync.dma_start(out=svr2[5:6, PW:PW + VW], in_=sv_hi)
                hl = PAIRS[p][1]
                vpairs = []
                for fg in range(2):
                    vpair = vprime.tile([128, NMC, 2, PW], dt.bfloat16,
                                        tag=f"vpair{fg}")
                    for fl in range(2):
                        fi = 2 * fg + fl
                        for mc in range(NMC):
                            col = (hl * 4 + fi) * 8 + mc
                            nc.vector.tensor_scalar(
                                out=vpair[:, mc, fl, :], in0=vp_s[:, mc, :],
                                scalar1=ams_t[:, col:col + 1], scalar2=None,
                                op0=op.mult)
                    vpairs.append(vpair)
                return mag, vpairs, svr2

            def av_fg(p, hl, mag, vpairs, svr2, acc, fg):
                """P = mag.T @ V'pair; o = (P+aug)/rs-col; acc += o."""
                vpair = vpairs[fg]
                for lc in range(NLC):
                    o_ps = ps_av.tile([128, 2 * PW], dt.float32, tag="ofp")
                    idx = (hl * 2 + fg) * 8 + lc
                    nc.tensor.matmul(
                        o_ps, aiv_t[:, idx * 128:(idx + 1) * 128],
                        svr2, start=True, stop=False)
                    for mc in range(NMC):
                        nc.tensor.matmul(
                            o_ps, mag[:, mc, lc * 128:(lc + 1) * 128],
                            vpair[:, mc, :, :],
                            start=False, stop=(mc == NMC - 1))
                    for fl in range(2):
                        fi = 2 * fg + fl
                        base = fl * PW
                        rs = rspool.tile([128, 1], dt.float32)
                        nc.vector.reciprocal(
                            rs, o_ps[:, base + 2 * D:base + 2 * D + 1])
                        if fi == 0:
                            nc.vector.tensor_scalar(
                                out=acc[:, lc, :],
                                in0=o_ps[:, base:base + 2 * D],
                                scalar1=rs, scalar2=None, op0=op.mult)
                        else:
                            nc.vector.scalar_tensor_tensor(
                                out=acc[:, lc, :],
                                in0=o_ps[:, base:base + 2 * D],
                                scalar=rs, in1=acc[:, lc, :],
                                op0=op.mult, op1=op.add)

            def expert_out(p, acc):
                # expert pattern complex multiply + store
                for lc in range(NLC):
                    p1 = ppool.tile([128, 128], dt.float32, tag="p1")
                    nc.gpsimd.tensor_tensor(p1, acc[:, lc, :],
                                            epq_t[:, lc, 0:128], op.mult)
                    p2 = ppool.tile([128, 128], dt.float32, tag="p2")
                    nc.gpsimd.tensor_tensor(p2, acc[:, lc, :],
                                            epq_t[:, lc, 128:256], op.mult)
                    o_r = outpool.tile([128, D], dt.float32, tag="o_r")
                    nc.vector.tensor_tensor(o_r, p1[:, 0:D], p1[:, D:2 * D],
                                            op.subtract)
                    o_i = outpool.tile([128, D], dt.float32, tag="o_i")
                    nc.vector.tensor_tensor(o_i, p2[:, 0:D], p2[:, D:2 * D],
                                            op.add)
                    nc.sync.dma_start(out=out_d[p, 0, lc], in_=o_r)
                    nc.sync.dma_start(out=out_d[p, 1, lc], in_=o_i)

            # software pipeline: scores/mag of pair p+1 are emitted before
            # the AV halves of pair p so ACT stays busy across pairs
            staged = load_and_scores(0)
            for p, (b, hl) in enumerate(PAIRS):
                cur = staged
                if p + 1 < len(PAIRS):
                    staged = load_and_scores(p + 1)
                acc = accpool.tile([128, NLC, 128], dt.float32)
                av_fg(p, hl, cur[0], cur[1], cur[2], acc, 0)
                av_fg(p, hl, cur[0], cur[1], cur[2], acc, 1)
                expert_out(p, acc)

    nc.compile()
    return nc


def get_module():
    if "nc" not in _module_cache:
        _module_cache["nc"] = _build_module()
    return _module_cache["nc"]


# ---------------------------------------------------------------- host driver
def make_in_maps(Q_real, Q_imag, K_real, K_imag, V_real, V_imag):
    A = _scale_abs()                      # [4, H, L]
    epq = _expert_quad()                  # [128, NLC, 256]
    ones = np.ones((L, 1), np.float32)
    in_maps = []
    for c in range(N_CORES):
        qa = np.empty((4, 128, L), BF16)
        qb = np.empty((4, 128, L), BF16)
        kt = np.empty((4, 128, L), BF16)
        vp = np.zeros((4, 128, NMC, PW), BF16)
        vf = np.empty((4, 128, NMC, VW), np.float32)
        aiv = np.zeros((6, 4096), BF16)
        ams = np.empty((128, 64), np.float32)
        for p, (b, hl) in enumerate(PAIRS):
            h = 2 * c + hl
            qrt = Q_real[b, h].T
            qit = Q_imag[b, h].T
            qa[p] = np.concatenate([qrt, -qit], 0).astype(BF16)
            qb[p] = np.concatenate([qit, qrt], 0).astype(BF16)
            kt[p] = np.concatenate([K_real[b, h].T, K_imag[b, h].T], 0).astype(BF16)
            vpack = np.concatenate([V_real[b, h], V_imag[b, h], ones], 1)
            vpack = vpack.reshape(NMC, 128, VW).transpose(1, 0, 2)
            vp[p, :, :, :VW] = vpack.astype(BF16)
            vf[p] = vpack
        for hl in range(2):
            h = 2 * c + hl
            for fi in range(4):
                am = (A[fi, h] / 8.0).reshape(NMC, 128).T  # [128, NMC]
                ams[:, (hl * 4 + fi) * 8:(hl * 4 + fi) * 8 + 8] = am
                ai = (1.0 / A[fi, h]).astype(np.float32)
                ai_hi = ai.astype(BF16)
                ai_lo = (ai - ai_hi.astype(np.float32)).astype(BF16)
                fg, fl = fi // 2, fi % 2
                base = (hl * 2 + fg) * 8 * 128
                aiv[3 * fl + 0, base:base + L] = ai_hi
                aiv[3 * fl + 1, base:base + L] = ai_hi
                aiv[3 * fl + 2, base:base + L] = ai_lo
        in_maps.append({"qa": qa, "qb": qb, "kt": kt, "vp": vp, "vf": vf,
                        "aiv": aiv, "ams": ams, "epq": epq})
    return in_maps


def gather_output(results):
    out = np.empty((2, B, H, L, D), np.float32)
    for c in range(N_CORES):
        o = results[c]["out"]  # [4, 2, NLC, 128, D]
        for p, (b, hl) in enumerate(PAIRS):
            h = 2 * c + hl
            out[0, b, h] = o[p, 0].reshape(L, D)
            out[1, b, h] = o[p, 1].reshape(L, D)
    return out


def kernel(**inputs):
    import time
    from concourse import bass_utils
    nc = get_module()
    in_maps = make_in_maps(**{k: np.asarray(v, np.float32) for k, v in inputs.items()})
    last = None
    for attempt in range(3):
        try:
            res = bass_utils.run_bass_kernel_spmd(
                nc, in_maps, core_ids=list(range(N_CORES)))
            return gather_output(res.results)
        except Exception as e:  # transient NRT_EXEC_UNIT_UNRECOVERABLE
            last = e
            time.sleep(2.0)
    raise last


if __name__ == "__main__":
    nc = get_module()
    print("module built OK")



# revision 3
# speedup vs baseline: 1.4978x; 1.4978x over previous
"""Trainium2 Bass kernel for nn_EnhancedQuantumLLM — fp8 DoubleRow redesign.

Math (B=2, H=16, L=1024, D=64):
  Per-scale pattern multiply is a per-(h,l) complex scalar c_l, so
  S = c_l c_m S0 with S0 = Q @ K^T (complex).  mag = |c_l||c_m||S0|/8.
  exp(x) = 1 + x to ~1e-7, softmax(w) = (1+x)/(L + sum x).  Dividing
  numerator and denominator by a_l/VS:
    num'[l,d] = VS*ai_l*sv[d] + P[l,d],   P = sum_m mag * av
    den'[l]   = VS*L*ai_l + r[l],         r = sum_m mag * aw
  with av = (a_m/8*VS)*[Vr|Vi], aw = a_m/8*VS, ai = 1/a, sv = colsum V.

Device pipeline per (b,h) pair:
  - scores: fp8e4 DoubleRow matmuls -> ps_r/ps_i [128,1024] PSUM (S0^T)
  - mag: ACT t1=Square(ps_r); DVE u=(ps_i pow 2)+t1 (bf16); then
    m-chunks 0-3 -> fp8 (ACT Sqrt x2, DVE magic-sqrt + Pool copy x2),
    m-chunks 4-7 -> bf16 (DVE magic-sqrt), consumed by bf16 matmuls.
  - AV per l-chunk: seed matmul (bf16 K=12, 3-term hi/lo ai x sv), 2 fp8
    DoubleRow + 4 bf16 matmuls -> ps_N [128,512] (4 scales x [Vr|Vi]),
    ps_W [128,4] (denominators).
  - DVE recip; ACT drains ps_N to fp16; DVE 4x tensor-scalar chain
    multiplies each scale block by its 1/den and accumulates; Pool does
    the expert complex multiply; DMA out fp32.

Sharding: 32 (b,h) over 8 cores; core c owns h in {2c, 2c+1}, b in {0,1}.
"""
import sys

for _p in ("/opt/trn_rl_repo",):
    if _p not in sys.path:
        sys.path.insert(0, _p)

import numpy as np
import ml_dtypes

B, H, L, D = 2, 16, 1024, 64
LMAX = 2048
PI = float(np.pi)
N_CORES = 8
PAIRS = [(0, 0), (0, 1), (1, 0), (1, 1)]  # (b, h_local)
NMC = L // 128   # m-chunks
NLC = L // 128   # l-chunks
NF8 = 4          # m-chunks 0..NF8-1 run fp8 DoubleRow; rest bf16
VS = 16.0        # power-of-2 scale folded into av/aw/seeds
BF16 = ml_dtypes.bfloat16
F8 = ml_dtypes.float8_e4m3
# bf16 magic sqrt: sqrt(x) ~ bits(x)>>1 + MAGIC (max rel err ~3.9%)
MAGIC_BF16 = 0x1FBC
# scale for shift-only magic sqrt (bf16 AV chunks): sqrt(u) ~ bits(u)>>1 times FOLD
FOLD = 1.26899973e+19

_module_cache = {}


# ---------------------------------------------------------------- host math
def _scale_abs():
    """|c^f[h,l]| for the 4 scale freqs, [4, H, L]."""
    out = np.empty((4, H, L), np.float64)
    for fi, freq in enumerate([1.0, 0.5, 0.25, 0.1]):
        phase = 2.0 * PI * np.arange(H, dtype=np.float64) / H
        t = np.linspace(0.0, 2.0 * PI * freq, LMAX)
        a1 = t[None, :] + phase[:, None]
        a2 = 2.0 * t[None, :] + phase[:, None]
        a3 = 0.5 * t[None, :] + phase[:, None]
        pr = np.cos(a1) + np.cos(a2) + np.cos(a3)
        pi_ = np.sin(a1) + np.sin(a2) + np.sin(a3)
        norm = np.sqrt(np.sum(pr * pr + pi_ * pi_, axis=1, keepdims=True))
        pr, pi_ = pr / norm, pi_ / norm
        out[fi] = np.sqrt(pr * pr + pi_ * pi_)[:, :L]
    return out.astype(np.float32)


def _expert_quad():
    """[128, NLC, 256] bf16: [epr|epi|epi|epr] per l-chunk, x0.5 folded."""
    freqs = np.array([[0.3 + 0.1 * i, 0.2 + 0.1 * i, 0.1 + 0.1 * i]
                      for i in range(8)], np.float64).reshape(-1)
    t = np.linspace(0.0, 2.0 * PI, LMAX)
    phase_d = 2.0 * PI * np.arange(D, dtype=np.float64) / D
    ang = freqs[:, None, None] * t[None, :, None] + phase_d[None, None, :]
    col_norm = 1.0 / np.sqrt(float(LMAX))
    denom = np.sqrt(3.0) * np.sqrt(8.0)
    epr = (np.sum(np.cos(ang), axis=0) * (col_norm / denom))[:L] * 0.5
    epi = (np.sum(np.sin(ang), axis=0) * (col_norm / denom))[:L] * 0.5
    # [epr | epi | epi | -epr]: lets the device form o_r and o_i with a
    # single subtract over the stacked products
    quad = np.concatenate([epr, epi, epi, -epr], axis=1)  # [L, 256]
    return np.ascontiguousarray(
        quad.reshape(NLC, 128, 4 * D).transpose(1, 0, 2)).astype(BF16)


# ---------------------------------------------------------------- device code
def _build_module():
    import concourse.bacc as bacc
    import concourse.tile as tile
    from concourse import mybir

    dt = mybir.dt
    op = mybir.AluOpType
    AF = mybir.ActivationFunctionType
    DR = mybir.MatmulPerfMode.DoubleRow
    NB = L // 256  # DoubleRow m-chunk pairs among fp8 chunks

    nc = bacc.Bacc("TRN2", target_bir_lowering=False, debug=False,
                   num_devices=N_CORES)

    # per-pair score inputs, one tensor: [pair, 64, {kt,qa,qb}, 2, L] fp8
    qkt_d = nc.dram_tensor("qkt", [4, 64, 3, 2, L], dt.float8e4,
                           kind="ExternalInput").ap()
    # AV rhs: fp8 pairs for m-chunks 0..3, bf16 for m-chunks 4..7
    vq_d = nc.dram_tensor("vq", [4, 128, NF8 // 2, 2, 520], dt.float8e4,
                          kind="ExternalInput").ap()
    vb_d = nc.dram_tensor("vb", [4, 128, NMC - NF8, 520], dt.bfloat16,
                          kind="ExternalInput").ap()
    # seeds: aiv lhsT [12, 2(hl), L]; seedR [12, 4(p), 512]; seedW [12, 4(p), 4]
    aiv_d = nc.dram_tensor("aiv", [12, 2, L], dt.bfloat16, kind="ExternalInput").ap()
    sdr_d = nc.dram_tensor("sdr", [12, 4, 512], dt.bfloat16, kind="ExternalInput").ap()
    sdw_d = nc.dram_tensor("sdw", [12, 4, 4], dt.bfloat16, kind="ExternalInput").ap()
    # expert quad with 1/g folded per hl: [128, 2, NLC, 256]
    epq_d = nc.dram_tensor("epq", [128, 2, NLC, 256], dt.float16, kind="ExternalInput").ap()
    # per-l drain scale g (bounds fp16 drain): [128, 2(hl), NLC]
    gsc_d = nc.dram_tensor("gsc", [128, 2, NLC], dt.float32, kind="ExternalInput").ap()
    out_d = nc.dram_tensor("out", [4, NLC, 128, 2 * D], dt.float32,
                           kind="ExternalOutput").ap()

    with tile.TileContext(nc) as tc:
        with (
            tc.tile_pool(name="singles", bufs=1) as singles,
            tc.tile_pool(name="qk", bufs=2) as qk,
            tc.tile_pool(name="vpool", bufs=2) as vpool,
            tc.tile_pool(name="tpool", bufs=5) as tpool,
            tc.tile_pool(name="upool", bufs=5) as upool,
            tc.tile_pool(name="mbpool", bufs=5) as mbpool,
            tc.tile_pool(name="magpool", bufs=2) as magpool,
            tc.tile_pool(name="rspool", bufs=4) as rspool,
            tc.tile_pool(name="nbpool", bufs=6) as nbpool,
            tc.tile_pool(name="cpool", bufs=8) as cpool,
            tc.tile_pool(name="ppool", bufs=8) as ppool,
            tc.tile_pool(name="outpool", bufs=8) as outpool,
            tc.tile_pool(name="ps_sc", bufs=2, space="PSUM") as ps_sc,
            tc.tile_pool(name="ps_av", bufs=2, space="PSUM") as ps_av,
        ):
            def load_singles():
                aiv_t = singles.tile([12, 2, L], dt.bfloat16)
                nc.sync.dma_start(out=aiv_t, in_=aiv_d)
                sdr_t = singles.tile([12, 4, 512], dt.bfloat16)
                nc.sync.dma_start(out=sdr_t, in_=sdr_d)
                sdw_t = singles.tile([12, 4, 4], dt.bfloat16)
                nc.sync.dma_start(out=sdw_t, in_=sdw_d)
                epq_t = singles.tile([128, 2, NLC, 256], dt.float16)
                nc.sync.dma_start(out=epq_t, in_=epq_d)
                gsc_t = singles.tile([128, 2, NLC], dt.float32)
                nc.sync.dma_start(out=gsc_t, in_=gsc_d)
                return aiv_t, sdr_t, sdw_t, epq_t, gsc_t

            def loads(p):
                """DMA all inputs for pair p; allocate its mag tiles."""
                qkt_s = qk.tile([64, 3, 2, L], dt.float8e4, tag="qkt_s")
                # two halves so the first score matmuls can start sooner
                nc.sync.dma_start(out=qkt_s[:, :, :, 0:512],
                                  in_=qkt_d[p][:, :, :, 0:512])
                nc.sync.dma_start(out=qkt_s[:, :, :, 512:L],
                                  in_=qkt_d[p][:, :, :, 512:L])
                vq_s = vpool.tile([128, NF8 // 2, 2, 520], dt.float8e4, tag="vq_s")
                nc.sync.dma_start(out=vq_s, in_=vq_d[p])
                vb_s = vpool.tile([128, NMC - NF8, 520], dt.bfloat16, tag="vb_s")
                nc.sync.dma_start(out=vb_s, in_=vb_d[p])
                mag8 = magpool.tile([128, NF8, L], dt.float8e4, tag="mag8")
                magb = magpool.tile([128, NMC - NF8, L], dt.uint16, tag="magb")
                obuf = outpool.tile([128, NLC, 2 * D], dt.float32, tag="obuf")
                return dict(kt=qkt_s[:, 0], qa=qkt_s[:, 1], qb=qkt_s[:, 2],
                            vq=vq_s, vb=vb_s, mag8=mag8, magb=magb,
                            obuf=obuf)

            def scores_hmc(st, mc, h):
                """fp8 DoubleRow scores for one m-chunk l-half; mag out.

                ps_ph packs S0r (cols 0:512) and S0i (cols 512:1024) for l in
                [h*512, (h+1)*512) so one ACT Square drains both components;
                half-width tiles double-buffer within 8 PSUM banks."""
                ps_ph = ps_sc.tile([128, L], dt.float32, tag="ps_ph")
                lhs = st["kt"][:, :, mc * 128:(mc + 1) * 128]
                sl = slice(h * 512, (h + 1) * 512)
                nc.tensor.matmul(ps_ph[:, 0:512], lhs, st["qa"][:, :, sl],
                                 start=True, stop=True, perf_mode=DR)
                nc.tensor.matmul(ps_ph[:, 512:1024], lhs, st["qb"][:, :, sl],
                                 start=True, stop=True, perf_mode=DR)
                t12 = tpool.tile([128, L], dt.bfloat16, tag="t12")
                nc.scalar.activation(t12, ps_ph, AF.Square)
                u = upool.tile([128, 512], dt.uint16, tag="u")
                nc.vector.tensor_tensor(u.bitcast(dt.bfloat16), t12[:, 0:512],
                                        t12[:, 512:1024], op.add)
                # magic sqrt: bits>>1 (+MAGIC for the fp8 chunks; two
                # single-op passes since bitwise/arith can't mix in one op)
                if mc < NF8:
                    mb16 = mbpool.tile([128, 512], dt.uint16, tag="mb16")
                    nc.vector.tensor_scalar(
                        out=mb16, in0=u, scalar1=1, scalar2=None,
                        op0=op.logical_shift_right)
                    nc.vector.tensor_scalar(
                        out=mb16, in0=mb16, scalar1=MAGIC_BF16, scalar2=None,
                        op0=op.add)
                    nc.gpsimd.tensor_copy(out=st["mag8"][:, mc, sl],
                                          in_=mb16.bitcast(dt.bfloat16))
                else:
                    # shift-only sqrt: decodes to sqrt(u)*2^-63.46; the
                    # 2^63.46 factor is folded into vb on the host
                    nc.vector.tensor_scalar(
                        out=st["magb"][:, mc - NF8, sl], in0=u,
                        scalar1=1, scalar2=None, op0=op.logical_shift_right)

            ps_w8_box = {}

            def av_lc(p, hl, st, lc):
                """Seeded mixed fp8/bf16 AV + normalize + expert + store."""
                lsl = slice(lc * 128, (lc + 1) * 128)
                ps_n = ps_av.tile([128, 512], dt.float32, tag="ps_n")
                if lc % 2 == 0:
                    ps_w8_box["t"] = ps_av.tile([128, 8], dt.float32,
                                                tag="ps_w8", name="ps_w8")
                ps_w8 = ps_w8_box["t"]
                wb = 4 * (lc % 2)
                ps_w = ps_w8[:, wb:wb + 4]
                aiv_l = aiv_t[:, hl, lsl]
                nc.tensor.matmul(ps_n, aiv_l, sdr_t[:, p, :],
                                 start=True, stop=False)
                nc.tensor.matmul(ps_w, aiv_l, sdw_t[:, p, :],
                                 start=True, stop=False)
                magb_bf = st["magb"].bitcast(dt.bfloat16)
                for mcp in range(NF8 // 2):
                    lhs = st["mag8"][:, 2 * mcp:2 * mcp + 2, lsl]
                    nc.tensor.matmul(ps_n, lhs, st["vq"][:, mcp, :, 0:512],
                                     start=False, stop=False, perf_mode=DR)
                    nc.tensor.matmul(ps_w, lhs, st["vq"][:, mcp, :, 512:516],
                                     start=False, stop=False, perf_mode=DR)
                for mb in range(NMC - NF8):
                    last = mb == NMC - NF8 - 1
                    lhs = magb_bf[:, mb, lsl]
                    nc.tensor.matmul(ps_n, lhs, st["vb"][:, mb, 0:512],
                                     start=False, stop=last)
                    nc.tensor.matmul(ps_w, lhs, st["vb"][:, mb, 512:516],
                                     start=False, stop=last)
                g_l = gsc_t[:, hl, lc:lc + 1]
                # one reciprocal per l-chunk pair (both halves of ps_w8);
                # per-chunk for the last pair to shorten the tail chain
                if p == 3:
                    rs4 = rspool.tile([128, 4], dt.float32, tag="rs4")
                    nc.vector.reciprocal(rs4, ps_w)
                elif lc % 2 == 1:
                    rs8 = rspool.tile([128, 8], dt.float32, tag="rs8")
                    nc.vector.reciprocal(rs8, ps_w8)
                    ps_w8_box["rs"] = rs8
                # ACT drains g*ps_N to fp16; DVE 4x ops apply 1/den per
                # scale block in two parallel 2-chains + one join; the 1/g
                # is folded into the expert quad on host.
                nb = nbpool.tile([128, 512], dt.float16, tag="nb")
                if lc < 3 or p == 3:
                    nc.scalar.activation(nb, ps_n, AF.Copy, scale=g_l)
                else:
                    nc.vector.tensor_scalar(out=nb, in0=ps_n, scalar1=g_l,
                                            scalar2=None, op0=op.mult)

                def combine_expert(lc, nb, rs8, wb):
                    c0 = cpool.tile([128, 128], dt.float16, tag="c0")
                    nc.vector.tensor_scalar(
                        out=c0, in0=nb[:, 0:128], scalar1=rs8[:, wb:wb + 1],
                        scalar2=None, op0=op.mult)
                    c1 = cpool.tile([128, 128], dt.float16, tag="c1")
                    nc.vector.scalar_tensor_tensor(
                        out=c1, in0=nb[:, 128:256], scalar=rs8[:, wb + 1:wb + 2],
                        in1=c0, op0=op.mult, op1=op.add)
                    c2 = cpool.tile([128, 128], dt.float16, tag="c2")
                    nc.vector.tensor_scalar(
                        out=c2, in0=nb[:, 256:384], scalar1=rs8[:, wb + 2:wb + 3],
                        scalar2=None, op0=op.mult)
                    acc = cpool.tile([128, 128], dt.float16, tag="acc")
                    nc.vector.scalar_tensor_tensor(
                        out=acc, in0=nb[:, 384:512], scalar=rs8[:, wb + 3:wb + 4],
                        in1=c2, op0=op.mult, op1=op.add)
                    accj = cpool.tile([128, 128], dt.float16, tag="accj")
                    nc.vector.tensor_tensor(accj, c1, acc, op.add)
                    # expert complex multiply on Pool: p12 = acc x quad,
                    # then one subtract (epq holds [epr|epi|epi|-epr])
                    p12 = ppool.tile([128, 2, 128], dt.float16, tag="p12")
                    nc.gpsimd.tensor_tensor(p12[:, 0, :], accj,
                                            epq_t[:, hl, lc, 0:128], op.mult)
                    nc.gpsimd.tensor_tensor(p12[:, 1, :], accj,
                                            epq_t[:, hl, lc, 128:256], op.mult)
                    obuf = st["obuf"]
                    ov = obuf[:, lc, 0:2 * D]
                    nc.gpsimd.tensor_tensor(
                        ov.rearrange("p (two d) -> p two d", two=2),
                        p12[:, :, 0:D], p12[:, :, D:2 * D], op.subtract)

                if p == 3:
                    combine_expert(lc, nb, rs4, 0)
                elif lc % 2 == 1:
                    combine_expert(lc - 1, ps_w8_box.pop("nb_prev"),
                                   ps_w8_box["rs"], 0)
                    combine_expert(lc, nb, ps_w8_box["rs"], 4)
                else:
                    ps_w8_box["nb_prev"] = nb
                if p == 3:
                    # quarter DMAs: the final transfer leaves earlier
                    if lc % 2 == 1:
                        nc.sync.dma_start(
                            out=out_d[p, lc - 1:lc + 1].transpose([1, 0, 2]),
                            in_=st["obuf"][:, lc - 1:lc + 1, :])
                elif lc == NLC // 2 - 1:
                    nc.sync.dma_start(out=out_d[p, 0:NLC // 2].transpose([1, 0, 2]),
                                      in_=st["obuf"][:, 0:NLC // 2, :])
                elif lc == NLC - 1:
                    nc.sync.dma_start(out=out_d[p, NLC // 2:NLC].transpose([1, 0, 2]),
                                      in_=st["obuf"][:, NLC // 2:NLC, :])

            # software pipeline with fine-grained interleave: scores m-chunk
            # i of pair p+1 is emitted right before AV l-chunk i of pair p,
            # so every engine alternates between the two pairs' work.
            # pair-0 input DMA goes first so PE starts ASAP; the large
            # constant tables follow behind it in the queue.
            staged = loads(0)
            aiv_t, sdr_t, sdw_t, epq_t, gsc_t = load_singles()
            for mc in range(NMC):
                scores_hmc(staged, mc, 0)
                scores_hmc(staged, mc, 1)
            for p, (b, hl) in enumerate(PAIRS):
                cur = staged
                if p + 1 < len(PAIRS):
                    staged = loads(p + 1)
                for i in range(NLC):
                    if p + 1 < len(PAIRS):
                        scores_hmc(staged, i, 0)
                        scores_hmc(staged, i, 1)
                    av_lc(p, hl, cur, i)

    nc.compile()
    return nc


def get_module():
    if "nc" not in _module_cache:
        _module_cache["nc"] = _build_module()
    return _module_cache["nc"]


# ---------------------------------------------------------------- host driver
def make_in_maps(Q_real, Q_imag, K_real, K_imag, V_real, V_imag):
    A = _scale_abs()                      # [4, H, L]
    epq_base = _expert_quad().astype(np.float32)   # [128, NLC, 256]
    in_maps = []
    for c in range(N_CORES):
        qkt = np.empty((4, 64, 3, 2, L), F8)
        kt, qa, qb = qkt[:, :, 0], qkt[:, :, 1], qkt[:, :, 2]
        vq = np.zeros((4, 128, NF8 // 2, 2, 520), F8)
        vb = np.zeros((4, 128, NMC - NF8, 520), BF16)
        sdr = np.zeros((12, 4, 512), BF16)
        sdw = np.zeros((12, 4, 4), BF16)
        aiv = np.zeros((12, 2, L), BF16)
        gsc = np.empty((128, 2, NLC), np.float32)
        epq = np.empty((128, 2, NLC, 256), np.float16)
        for p, (b, hl) in enumerate(PAIRS):
            h = 2 * c + hl
            qa[p, :, 0, :] = Q_real[b, h].T.astype(F8)
            qa[p, :, 1, :] = (-Q_imag[b, h].T).astype(F8)
            qb[p, :, 0, :] = Q_imag[b, h].T.astype(F8)
            qb[p, :, 1, :] = Q_real[b, h].T.astype(F8)
            kt[p, :, 0, :] = K_real[b, h].T.astype(F8)
            kt[p, :, 1, :] = K_imag[b, h].T.astype(F8)
            # AV rhs: per m: [av (4 scales x 128) | aw (4) | pad]
            vcat = np.concatenate([V_real[b, h], V_imag[b, h]], 1)  # [L, 128]
            av = np.zeros((L, 520), np.float32)
            for fi in range(4):
                am = (A[fi, h] / 8.0) * VS                       # [L]
                av[:, fi * 128:(fi + 1) * 128] = am[:, None] * vcat
                av[:, 512 + fi] = am
            avc = av.reshape(NMC, 128, 520)    # [m-chunk, m-part, 520]
            vq[p] = np.ascontiguousarray(
                avc[:NF8].reshape(NF8 // 2, 2, 128, 520)
                .transpose(2, 0, 1, 3)).astype(F8)
            vb[p] = np.ascontiguousarray(
                avc[NF8:].transpose(1, 0, 2) * FOLD).astype(BF16)
            # seeds
            sv = vcat.sum(0, dtype=np.float32) * VS              # [128]
            sv_hi = sv.astype(BF16)
            sv_lo = (sv - sv_hi.astype(np.float32)).astype(BF16)
            for fi in range(4):
                sdr[3 * fi + 0, p, fi * 128:(fi + 1) * 128] = sv_hi
                sdr[3 * fi + 1, p, fi * 128:(fi + 1) * 128] = sv_lo
                sdr[3 * fi + 2, p, fi * 128:(fi + 1) * 128] = sv_hi
                sdw[3 * fi + 0, p, fi] = np.float32(VS * L)
                sdw[3 * fi + 2, p, fi] = np.float32(VS * L)
        for hl in range(2):
            h = 2 * c + hl
            for fi in range(4):
                ai = (1.0 / A[fi, h]).astype(np.float32)
                ai_hi = ai.astype(BF16)
                ai_lo = (ai - ai_hi.astype(np.float32)).astype(BF16)
                aiv[3 * fi + 0, hl] = ai_hi
                aiv[3 * fi + 1, hl] = ai_hi
                aiv[3 * fi + 2, hl] = ai_lo
            g = 8.0 * A[:, h].min(axis=0) / VS               # [L]
            gm = g.reshape(NLC, 128).T                       # [128, NLC]
            gsc[:, hl, :] = gm
            epq[:, hl] = (epq_base / gm[:, :, None]).astype(np.float16)
        in_maps.append({"qkt": qkt, "vq": vq, "vb": vb,
                        "aiv": aiv, "sdr": sdr, "sdw": sdw, "epq": epq,
                        "gsc": gsc})
    return in_maps


def gather_output(results):
    out = np.empty((2, B, H, L, D), np.float32)
    for c in range(N_CORES):
        o = results[c]["out"]  # [4, NLC, 128, 2*D]
        for p, (b, hl) in enumerate(PAIRS):
            h = 2 * c + hl
            out[0, b, h] = o[p, :, :, 0:D].reshape(L, D)
            out[1, b, h] = o[p, :, :, D:2 * D].reshape(L, D)
    return out


def kernel(**inputs):
    import time
    from concourse import bass_utils
    nc = get_module()
    in_maps = make_in_maps(**{k: np.asarray(v, np.float32) for k, v in inputs.items()})
    last = None
    for attempt in range(3):
        try:
            res = bass_utils.run_bass_kernel_spmd(
                nc, in_maps, core_ids=list(range(N_CORES)))
            return gather_output(res.results)
        except Exception as e:  # transient NRT_EXEC_UNIT_UNRECOVERABLE
            last = e
            time.sleep(2.0)
    raise last


if __name__ == "__main__":
    nc = get_module()
    print("module built OK")


# revision 4
# speedup vs baseline: 1.5108x; 1.0087x over previous
"""Trainium2 Bass kernel for nn_EnhancedQuantumLLM — fp8 DoubleRow redesign.

Math (B=2, H=16, L=1024, D=64):
  Per-scale pattern multiply is a per-(h,l) complex scalar c_l, so
  S = c_l c_m S0 with S0 = Q @ K^T (complex).  mag = |c_l||c_m||S0|/8.
  exp(x) = 1 + x to ~1e-7, softmax(w) = (1+x)/(L + sum x).  Dividing
  numerator and denominator by a_l/VS:
    num'[l,d] = VS*ai_l*sv[d] + P[l,d],   P = sum_m mag * av
    den'[l]   = VS*L*ai_l + r[l],         r = sum_m mag * aw
  with av = (a_m/8*VS)*[Vr|Vi], aw = a_m/8*VS, ai = 1/a, sv = colsum V.

Device pipeline per (b,h) pair:
  - scores: fp8e4 DoubleRow matmuls -> ps_r/ps_i [128,1024] PSUM (S0^T)
  - mag: ACT t1=Square(ps_r); DVE u=(ps_i pow 2)+t1 (bf16); then
    m-chunks 0-3 -> fp8 (ACT Sqrt x2, DVE magic-sqrt + Pool copy x2),
    m-chunks 4-7 -> bf16 (DVE magic-sqrt), consumed by bf16 matmuls.
  - AV per l-chunk: seed matmul (bf16 K=12, 3-term hi/lo ai x sv), 2 fp8
    DoubleRow + 4 bf16 matmuls -> ps_N [128,512] (4 scales x [Vr|Vi]),
    ps_W [128,4] (denominators).
  - DVE recip; ACT drains ps_N to fp16; DVE 4x tensor-scalar chain
    multiplies each scale block by its 1/den and accumulates; Pool does
    the expert complex multiply; DMA out fp32.

Sharding: 32 (b,h) over 8 cores; core c owns h in {2c, 2c+1}, b in {0,1}.
"""
import sys

for _p in ("/opt/trn_rl_repo",):
    if _p not in sys.path:
        sys.path.insert(0, _p)

import numpy as np
import ml_dtypes

B, H, L, D = 2, 16, 1024, 64
LMAX = 2048
PI = float(np.pi)
N_CORES = 8
PAIRS = [(0, 0), (0, 1), (1, 0), (1, 1)]  # (b, h_local)
NMC = L // 128   # m-chunks
NLC = L // 128   # l-chunks
NF8 = 4          # m-chunks 0..NF8-1 run fp8 DoubleRow; rest bf16
VS = 16.0        # power-of-2 scale folded into av/aw/seeds
BF16 = ml_dtypes.bfloat16
F8 = ml_dtypes.float8_e4m3
# bf16 magic sqrt: sqrt(x) ~ bits(x)>>1 + MAGIC (max rel err ~3.9%)
MAGIC_BF16 = 0x1FBC
# scale for shift-only magic sqrt (bf16 AV chunks): sqrt(u) ~ bits(u)>>1 times FOLD
FOLD = 1.26899973e+19

_module_cache = {}


# ---------------------------------------------------------------- host math
def _scale_abs():
    """|c^f[h,l]| for the 4 scale freqs, [4, H, L]."""
    out = np.empty((4, H, L), np.float64)
    for fi, freq in enumerate([1.0, 0.5, 0.25, 0.1]):
        phase = 2.0 * PI * np.arange(H, dtype=np.float64) / H
        t = np.linspace(0.0, 2.0 * PI * freq, LMAX)
        a1 = t[None, :] + phase[:, None]
        a2 = 2.0 * t[None, :] + phase[:, None]
        a3 = 0.5 * t[None, :] + phase[:, None]
        pr = np.cos(a1) + np.cos(a2) + np.cos(a3)
        pi_ = np.sin(a1) + np.sin(a2) + np.sin(a3)
        norm = np.sqrt(np.sum(pr * pr + pi_ * pi_, axis=1, keepdims=True))
        pr, pi_ = pr / norm, pi_ / norm
        out[fi] = np.sqrt(pr * pr + pi_ * pi_)[:, :L]
    return out.astype(np.float32)


def _expert_quad():
    """[128, NLC, 256] bf16: [epr|epi|epi|epr] per l-chunk, x0.5 folded."""
    freqs = np.array([[0.3 + 0.1 * i, 0.2 + 0.1 * i, 0.1 + 0.1 * i]
                      for i in range(8)], np.float64).reshape(-1)
    t = np.linspace(0.0, 2.0 * PI, LMAX)
    phase_d = 2.0 * PI * np.arange(D, dtype=np.float64) / D
    ang = freqs[:, None, None] * t[None, :, None] + phase_d[None, None, :]
    col_norm = 1.0 / np.sqrt(float(LMAX))
    denom = np.sqrt(3.0) * np.sqrt(8.0)
    epr = (np.sum(np.cos(ang), axis=0) * (col_norm / denom))[:L] * 0.5
    epi = (np.sum(np.sin(ang), axis=0) * (col_norm / denom))[:L] * 0.5
    # [epr | epi | epi | -epr]: lets the device form o_r and o_i with a
    # single subtract over the stacked products
    quad = np.concatenate([epr, epi, epi, -epr], axis=1)  # [L, 256]
    return np.ascontiguousarray(
        quad.reshape(NLC, 128, 4 * D).transpose(1, 0, 2)).astype(BF16)


# ---------------------------------------------------------------- device code
def _build_module():
    import concourse.bacc as bacc
    import concourse.tile as tile
    from concourse import mybir

    dt = mybir.dt
    op = mybir.AluOpType
    AF = mybir.ActivationFunctionType
    DR = mybir.MatmulPerfMode.DoubleRow
    NB = L // 256  # DoubleRow m-chunk pairs among fp8 chunks

    nc = bacc.Bacc("TRN2", target_bir_lowering=False, debug=False,
                   num_devices=N_CORES)

    # per-pair score inputs, one tensor: [pair, 64, {kt,qa,qb}, 2, L] fp8
    qkt_d = nc.dram_tensor("qkt", [4, 64, 3, 2, L], dt.float8e4,
                           kind="ExternalInput").ap()
    # AV rhs: fp8 pairs for m-chunks 0..3, bf16 for m-chunks 4..7
    vq_d = nc.dram_tensor("vq", [4, 128, NF8 // 2, 2, 520], dt.float8e4,
                          kind="ExternalInput").ap()
    vb_d = nc.dram_tensor("vb", [4, 128, NMC - NF8, 520], dt.bfloat16,
                          kind="ExternalInput").ap()
    # seeds: aiv lhsT [12, 2(hl), L]; seedR [12, 4(p), 512]; seedW [12, 4(p), 4]
    aiv_d = nc.dram_tensor("aiv", [12, 2, L], dt.bfloat16, kind="ExternalInput").ap()
    sdr_d = nc.dram_tensor("sdr", [12, 4, 512], dt.bfloat16, kind="ExternalInput").ap()
    sdw_d = nc.dram_tensor("sdw", [12, 4, 4], dt.bfloat16, kind="ExternalInput").ap()
    # expert quad with 1/g folded per hl: [128, 2, NLC, 256]
    epq_d = nc.dram_tensor("epq", [128, 2, NLC, 256], dt.float16, kind="ExternalInput").ap()
    # per-l drain scale g (bounds fp16 drain): [128, 2(hl), NLC]
    gsc_d = nc.dram_tensor("gsc", [128, 2, NLC], dt.float32, kind="ExternalInput").ap()
    out_d = nc.dram_tensor("out", [4, NLC, 128, 2 * D], dt.float32,
                           kind="ExternalOutput").ap()

    with tile.TileContext(nc) as tc:
        with (
            tc.tile_pool(name="singles", bufs=1) as singles,
            tc.tile_pool(name="qk", bufs=2) as qk,
            tc.tile_pool(name="vpool", bufs=2) as vpool,
            tc.tile_pool(name="tpool", bufs=5) as tpool,
            tc.tile_pool(name="upool", bufs=5) as upool,
            tc.tile_pool(name="mbpool", bufs=5) as mbpool,
            tc.tile_pool(name="magpool", bufs=2) as magpool,
            tc.tile_pool(name="rspool", bufs=4) as rspool,
            tc.tile_pool(name="nbpool", bufs=6) as nbpool,
            tc.tile_pool(name="cpool", bufs=8) as cpool,
            tc.tile_pool(name="ppool", bufs=8) as ppool,
            tc.tile_pool(name="outpool", bufs=8) as outpool,
            tc.tile_pool(name="ps_sc", bufs=2, space="PSUM") as ps_sc,
            tc.tile_pool(name="ps_av", bufs=2, space="PSUM") as ps_av,
        ):
            def load_singles():
                aiv_t = singles.tile([12, 2, L], dt.bfloat16)
                nc.sync.dma_start(out=aiv_t, in_=aiv_d)
                sdr_t = singles.tile([12, 4, 512], dt.bfloat16)
                nc.sync.dma_start(out=sdr_t, in_=sdr_d)
                sdw_t = singles.tile([12, 4, 4], dt.bfloat16)
                nc.sync.dma_start(out=sdw_t, in_=sdw_d)
                epq_t = singles.tile([128, 2, NLC, 256], dt.float16)
                nc.sync.dma_start(out=epq_t, in_=epq_d)
                gsc_t = singles.tile([128, 2, NLC], dt.float32)
                nc.sync.dma_start(out=gsc_t, in_=gsc_d)
                return aiv_t, sdr_t, sdw_t, epq_t, gsc_t

            def loads(p):
                """DMA all inputs for pair p; allocate its mag tiles."""
                qkt_s = qk.tile([64, 3, 2, L], dt.float8e4, tag="qkt_s")
                # two halves so the first score matmuls can start sooner
                nc.sync.dma_start(out=qkt_s[:, :, :, 0:512],
                                  in_=qkt_d[p][:, :, :, 0:512])
                nc.sync.dma_start(out=qkt_s[:, :, :, 512:L],
                                  in_=qkt_d[p][:, :, :, 512:L])
                vq_s = vpool.tile([128, NF8 // 2, 2, 520], dt.float8e4, tag="vq_s")
                nc.sync.dma_start(out=vq_s, in_=vq_d[p])
                vb_s = vpool.tile([128, NMC - NF8, 520], dt.bfloat16, tag="vb_s")
                nc.sync.dma_start(out=vb_s, in_=vb_d[p])
                mag8 = magpool.tile([128, NF8, L], dt.float8e4, tag="mag8")
                magb = magpool.tile([128, NMC - NF8, L], dt.uint16, tag="magb")
                obuf = outpool.tile([128, NLC, 2 * D], dt.float32, tag="obuf")
                return dict(kt=qkt_s[:, 0], qa=qkt_s[:, 1], qb=qkt_s[:, 2],
                            vq=vq_s, vb=vb_s, mag8=mag8, magb=magb,
                            obuf=obuf)

            def scores_hmc(st, mc, h):
                """fp8 DoubleRow scores for one m-chunk l-half; mag out.

                ps_ph packs S0r (cols 0:512) and S0i (cols 512:1024) for l in
                [h*512, (h+1)*512) so one ACT Square drains both components;
                half-width tiles double-buffer within 8 PSUM banks."""
                ps_ph = ps_sc.tile([128, L], dt.float32, tag="ps_ph")
                lhs = st["kt"][:, :, mc * 128:(mc + 1) * 128]
                sl = slice(h * 512, (h + 1) * 512)
                nc.tensor.matmul(ps_ph[:, 0:512], lhs, st["qa"][:, :, sl],
                                 start=True, stop=True, perf_mode=DR)
                nc.tensor.matmul(ps_ph[:, 512:1024], lhs, st["qb"][:, :, sl],
                                 start=True, stop=True, perf_mode=DR)
                t12 = tpool.tile([128, L], dt.bfloat16, tag="t12")
                nc.scalar.activation(t12, ps_ph, AF.Square)
                u = upool.tile([128, 512], dt.uint16, tag="u")
                nc.vector.tensor_tensor(u.bitcast(dt.bfloat16), t12[:, 0:512],
                                        t12[:, 512:1024], op.add)
                # magic sqrt: bits>>1 (+MAGIC for the fp8 chunks; two
                # single-op passes since bitwise/arith can't mix in one op)
                if mc < NF8:
                    mb16 = mbpool.tile([128, 512], dt.uint16, tag="mb16")
                    nc.vector.tensor_scalar(
                        out=mb16, in0=u, scalar1=1, scalar2=None,
                        op0=op.logical_shift_right)
                    nc.vector.tensor_scalar(
                        out=mb16, in0=mb16, scalar1=MAGIC_BF16, scalar2=None,
                        op0=op.add)
                    nc.gpsimd.tensor_copy(out=st["mag8"][:, mc, sl],
                                          in_=mb16.bitcast(dt.bfloat16))
                else:
                    # shift-only sqrt: decodes to sqrt(u)*2^-63.46; the
                    # 2^63.46 factor is folded into vb on the host
                    nc.vector.tensor_scalar(
                        out=st["magb"][:, mc - NF8, sl], in0=u,
                        scalar1=1, scalar2=None, op0=op.logical_shift_right)

            ps_w8_box = {}

            def av_lc(p, hl, st, lc):
                """Seeded mixed fp8/bf16 AV + normalize + expert + store."""
                lsl = slice(lc * 128, (lc + 1) * 128)
                ps_n = ps_av.tile([128, 512], dt.float32, tag="ps_n")
                if lc % 2 == 0:
                    ps_w8_box["t"] = ps_av.tile([128, 8], dt.float32,
                                                tag="ps_w8", name="ps_w8")
                ps_w8 = ps_w8_box["t"]
                wb = 4 * (lc % 2)
                ps_w = ps_w8[:, wb:wb + 4]
                aiv_l = aiv_t[:, hl, lsl]
                nc.tensor.matmul(ps_n, aiv_l, sdr_t[:, p, :],
                                 start=True, stop=False)
                nc.tensor.matmul(ps_w, aiv_l, sdw_t[:, p, :],
                                 start=True, stop=False)
                magb_bf = st["magb"].bitcast(dt.bfloat16)
                for mcp in range(NF8 // 2):
                    lhs = st["mag8"][:, 2 * mcp:2 * mcp + 2, lsl]
                    nc.tensor.matmul(ps_n, lhs, st["vq"][:, mcp, :, 0:512],
                                     start=False, stop=False, perf_mode=DR)
                    nc.tensor.matmul(ps_w, lhs, st["vq"][:, mcp, :, 512:516],
                                     start=False, stop=False, perf_mode=DR)
                for mb in range(NMC - NF8):
                    last = mb == NMC - NF8 - 1
                    lhs = magb_bf[:, mb, lsl]
                    nc.tensor.matmul(ps_n, lhs, st["vb"][:, mb, 0:512],
                                     start=False, stop=last)
                    nc.tensor.matmul(ps_w, lhs, st["vb"][:, mb, 512:516],
                                     start=False, stop=last)
                g_l = gsc_t[:, hl, lc:lc + 1]
                # one reciprocal per l-chunk pair (both halves of ps_w8);
                # per-chunk for the last pair to shorten the tail chain
                if p == 3:
                    rs4 = rspool.tile([128, 4], dt.float32, tag="rs4")
                    nc.vector.reciprocal(rs4, ps_w)
                elif lc % 2 == 1:
                    rs8 = rspool.tile([128, 8], dt.float32, tag="rs8")
                    nc.vector.reciprocal(rs8, ps_w8)
                    ps_w8_box["rs"] = rs8
                # ACT drains g*ps_N to fp16; DVE 4x ops apply 1/den per
                # scale block in two parallel 2-chains + one join; the 1/g
                # is folded into the expert quad on host.
                nb = nbpool.tile([128, 512], dt.float16, tag="nb")
                if lc < 3 or p == 3:
                    nc.scalar.activation(nb, ps_n, AF.Copy, scale=g_l)
                else:
                    nc.vector.tensor_scalar(out=nb, in0=ps_n, scalar1=g_l,
                                            scalar2=None, op0=op.mult)

                def combine_expert(lc, nb, rs8, wb):
                    c0 = cpool.tile([128, 128], dt.float16, tag="c0")
                    nc.vector.tensor_scalar(
                        out=c0, in0=nb[:, 0:128], scalar1=rs8[:, wb:wb + 1],
                        scalar2=None, op0=op.mult)
                    c1 = cpool.tile([128, 128], dt.float16, tag="c1")
                    nc.vector.scalar_tensor_tensor(
                        out=c1, in0=nb[:, 128:256], scalar=rs8[:, wb + 1:wb + 2],
                        in1=c0, op0=op.mult, op1=op.add)
                    c2 = cpool.tile([128, 128], dt.float16, tag="c2")
                    nc.vector.tensor_scalar(
                        out=c2, in0=nb[:, 256:384], scalar1=rs8[:, wb + 2:wb + 3],
                        scalar2=None, op0=op.mult)
                    acc = cpool.tile([128, 128], dt.float16, tag="acc")
                    nc.vector.scalar_tensor_tensor(
                        out=acc, in0=nb[:, 384:512], scalar=rs8[:, wb + 3:wb + 4],
                        in1=c2, op0=op.mult, op1=op.add)
                    accj = cpool.tile([128, 128], dt.float16, tag="accj")
                    nc.vector.tensor_tensor(accj, c1, acc, op.add)
                    # expert complex multiply: p12 = acc x quad, then one
                    # subtract (epq holds [epr|epi|epi|-epr]).  Pool does it
                    # in the bulk; the tail pair uses the otherwise-idle DVE
                    # to shorten the end-of-kernel chain.
                    p12 = ppool.tile([128, 2, 128], dt.float16, tag="p12")
                    nc.gpsimd.tensor_tensor(p12[:, 0, :], accj,
                                            epq_t[:, hl, lc, 0:128], op.mult)
                    nc.gpsimd.tensor_tensor(p12[:, 1, :], accj,
                                            epq_t[:, hl, lc, 128:256], op.mult)
                    obuf = st["obuf"]
                    ov = obuf[:, lc, 0:2 * D]
                    nc.gpsimd.tensor_tensor(
                        ov.rearrange("p (two d) -> p two d", two=2),
                        p12[:, :, 0:D], p12[:, :, D:2 * D], op.subtract)

                if p == 3:
                    combine_expert(lc, nb, rs4, 0)
                elif lc % 2 == 1:
                    combine_expert(lc - 1, ps_w8_box.pop("nb_prev"),
                                   ps_w8_box["rs"], 0)
                    combine_expert(lc, nb, ps_w8_box["rs"], 4)
                else:
                    ps_w8_box["nb_prev"] = nb
                if p == 3:
                    # quarter DMAs: the final transfer leaves earlier
                    if lc % 2 == 1:
                        nc.sync.dma_start(
                            out=out_d[p, lc - 1:lc + 1].transpose([1, 0, 2]),
                            in_=st["obuf"][:, lc - 1:lc + 1, :])
                elif lc == NLC // 2 - 1:
                    nc.sync.dma_start(out=out_d[p, 0:NLC // 2].transpose([1, 0, 2]),
                                      in_=st["obuf"][:, 0:NLC // 2, :])
                elif lc == NLC - 1:
                    nc.sync.dma_start(out=out_d[p, NLC // 2:NLC].transpose([1, 0, 2]),
                                      in_=st["obuf"][:, NLC // 2:NLC, :])

            # software pipeline with fine-grained interleave: scores m-chunk
            # i of pair p+1 is emitted right before AV l-chunk i of pair p,
            # so every engine alternates between the two pairs' work.
            # pair-0 input DMA goes first so PE starts ASAP; the large
            # constant tables follow behind it in the queue.
            staged = loads(0)
            aiv_t, sdr_t, sdw_t, epq_t, gsc_t = load_singles()
            for mc in range(NMC):
                scores_hmc(staged, mc, 0)
                scores_hmc(staged, mc, 1)
            for p, (b, hl) in enumerate(PAIRS):
                cur = staged
                if p + 1 < len(PAIRS):
                    staged = loads(p + 1)
                for i in range(NLC):
                    if p + 1 < len(PAIRS):
                        scores_hmc(staged, i, 0)
                    av_lc(p, hl, cur, i)
                    if p + 1 < len(PAIRS):
                        scores_hmc(staged, i, 1)

    nc.compile()
    return nc


def get_module():
    if "nc" not in _module_cache:
        _module_cache["nc"] = _build_module()
    return _module_cache["nc"]


# ---------------------------------------------------------------- host driver
def make_in_maps(Q_real, Q_imag, K_real, K_imag, V_real, V_imag):
    A = _scale_abs()                      # [4, H, L]
    epq_base = _expert_quad().astype(np.float32)   # [128, NLC, 256]
    in_maps = []
    for c in range(N_CORES):
        qkt = np.empty((4, 64, 3, 2, L), F8)
        kt, qa, qb = qkt[:, :, 0], qkt[:, :, 1], qkt[:, :, 2]
        vq = np.zeros((4, 128, NF8 // 2, 2, 520), F8)
        vb = np.zeros((4, 128, NMC - NF8, 520), BF16)
        sdr = np.zeros((12, 4, 512), BF16)
        sdw = np.zeros((12, 4, 4), BF16)
        aiv = np.zeros((12, 2, L), BF16)
        gsc = np.empty((128, 2, NLC), np.float32)
        epq = np.empty((128, 2, NLC, 256), np.float16)
        for p, (b, hl) in enumerate(PAIRS):
            h = 2 * c + hl
            qa[p, :, 0, :] = Q_real[b, h].T.astype(F8)
            qa[p, :, 1, :] = (-Q_imag[b, h].T).astype(F8)
            qb[p, :, 0, :] = Q_imag[b, h].T.astype(F8)
            qb[p, :, 1, :] = Q_real[b, h].T.astype(F8)
            kt[p, :, 0, :] = K_real[b, h].T.astype(F8)
            kt[p, :, 1, :] = K_imag[b, h].T.astype(F8)
            # AV rhs: per m: [av (4 scales x 128) | aw (4) | pad]
            vcat = np.concatenate([V_real[b, h], V_imag[b, h]], 1)  # [L, 128]
            av = np.zeros((L, 520), np.float32)
            for fi in range(4):
                am = (A[fi, h] / 8.0) * VS                       # [L]
                av[:, fi * 128:(fi + 1) * 128] = am[:, None] * vcat
                av[:, 512 + fi] = am
            avc = av.reshape(NMC, 128, 520)    # [m-chunk, m-part, 520]
            vq[p] = np.ascontiguousarray(
                avc[:NF8].reshape(NF8 // 2, 2, 128, 520)
                .transpose(2, 0, 1, 3)).astype(F8)
            vb[p] = np.ascontiguousarray(
                avc[NF8:].transpose(1, 0, 2) * FOLD).astype(BF16)
            # seeds
            sv = vcat.sum(0, dtype=np.float32) * VS              # [128]
            sv_hi = sv.astype(BF16)
            sv_lo = (sv - sv_hi.astype(np.float32)).astype(BF16)
            for fi in range(4):
                sdr[3 * fi + 0, p, fi * 128:(fi + 1) * 128] = sv_hi
                sdr[3 * fi + 1, p, fi * 128:(fi + 1) * 128] = sv_lo
                sdr[3 * fi + 2, p, fi * 128:(fi + 1) * 128] = sv_hi
                sdw[3 * fi + 0, p, fi] = np.float32(VS * L)
                sdw[3 * fi + 2, p, fi] = np.float32(VS * L)
        for hl in range(2):
            h = 2 * c + hl
            for fi in range(4):
                ai = (1.0 / A[fi, h]).astype(np.float32)
                ai_hi = ai.astype(BF16)
                ai_lo = (ai - ai_hi.astype(np.float32)).astype(BF16)
                aiv[3 * fi + 0, hl] = ai_hi
                aiv[3 * fi + 1, hl] = ai_hi
                aiv[3 * fi + 2, hl] = ai_lo
            g = 8.0 * A[:, h].min(axis=0) / VS               # [L]
            gm = g.reshape(NLC, 128).T                       # [128, NLC]
            gsc[:, hl, :] = gm
            epq[:, hl] = (epq_base / gm[:, :, None]).astype(np.float16)
        in_maps.append({"qkt": qkt, "vq": vq, "vb": vb,
                        "aiv": aiv, "sdr": sdr, "sdw": sdw, "epq": epq,
                        "gsc": gsc})
    return in_maps


def gather_output(results):
    out = np.empty((2, B, H, L, D), np.float32)
    for c in range(N_CORES):
        o = results[c]["out"]  # [4, NLC, 128, 2*D]
        for p, (b, hl) in enumerate(PAIRS):
            h = 2 * c + hl
            out[0, b, h] = o[p, :, :, 0:D].reshape(L, D)
            out[1, b, h] = o[p, :, :, D:2 * D].reshape(L, D)
    return out


def kernel(**inputs):
    import time
    from concourse import bass_utils
    nc = get_module()
    in_maps = make_in_maps(**{k: np.asarray(v, np.float32) for k, v in inputs.items()})
    last = None
    for attempt in range(3):
        try:
            res = bass_utils.run_bass_kernel_spmd(
                nc, in_maps, core_ids=list(range(N_CORES)))
            return gather_output(res.results)
        except Exception as e:  # transient NRT_EXEC_UNIT_UNRECOVERABLE
            last = e
            time.sleep(2.0)
    raise last


if __name__ == "__main__":
    nc = get_module()
    print("module built OK")
